# revision 16
# baseline (speedup 1.0000x reference)
"""Modulated conv2d (StyleGAN-2 style, B=16 C=128 HxW=128x128 K=3) on 8 TRN2
NeuronCores, data-parallel over batch (2 samples/core).

v2: the style modulation + demodulation (a ~3 MFLOP computation) is folded
into the weights ON HOST, so the device kernel is a pure grouped conv:

  per core:
    1. DMA: wmod[i, s, t*C+o] = (weight * style * demod)^T  (bf16, 0.59 MB)
            x zero-padded to 130x130 (bf16), in 4 chunks per sample with a
            small first chunk so the conv can start early
    2. warm-up: a few dummy matmuls on a memset scratch region keep the PE
       busy from kernel start so the HAM clock-gate reaches K=8/8 (2.4 GHz)
       before the real conv stream begins (otherwise the first ~3.4 us of
       matmuls run at 1.2 GHz)
    3. conv: per 4-row output block, 9 tap matmuls (K=C_in, M=C_out, N=512)
       accumulate fp32 in PSUM; the tap shift is a strided 3D rhs view into
       the padded image — no im2col; 7 rotating PSUM banks
    4. evict: psum -> sbuf staging copy on DVE (demod scale already in the
       weights), 12 staging buffers
    5. DMA out (ACT-issued HWDGE)

Raw Bass with manual semaphores: this toolchain's walrus accepts only ONE
sync-wait command per instruction, so every engine-pair dependency is guarded
by an explicit single-wait `wait_ge`.

Numerics: bf16 operands, fp32 accumulation; max rel err vs the fp32 jax
reference ~2.2e-3.
"""

import sys

sys.path.insert(0, "/opt/trn_rl_repo")

import numpy as np

import concourse.bass as bass
from concourse import mybir
from concourse.bass_utils import run_bass_kernel_spmd

B, C, H, W, KS, WD = 16, 128, 128, 128, 3, 512
NCORES = 8
SPC = B // NCORES          # samples per core = 2
HP = H + 2                 # padded height/width = 130
NT = KS * KS               # 9 taps

R = 4                      # output rows per conv block (N = R*W = 512; PSUM bank cap)
NPS = 7                    # rotating conv PSUM banks
NOB = 12                   # output staging buffers
NB = H // R                # conv blocks per sample = 32
NWARM = 5                  # PE warm-up matmuls (N=512 each)
CHUNK_BNDS = [0, 10, 18, 50, 90, 130]   # x DMA chunk row boundaries (padded rows)
NCH = len(CHUNK_BNDS) - 1


def _chunk_of_block(b):
    """First x chunk that covers padded rows needed by output block b."""
    need = R * b + R + 1
    for c in range(len(CHUNK_BNDS) - 1):
        if need < CHUNK_BNDS[c + 1]:
            return c
    raise AssertionError


F32 = mybir.dt.float32
BF16 = mybir.dt.bfloat16
MULT = mybir.AluOpType.mult


def build_program():
    nc = bass.Bass(trn_type="TRN2", target_bir_lowering=False, debug=False)

    xpad_d = nc.dram_tensor("xpad", [SPC, C, HP, HP], BF16, kind="ExternalInput").ap()
    wmod_d = nc.dram_tensor("wmod", [C, SPC, NT * C], BF16, kind="ExternalInput").ap()
    y_d = nc.dram_tensor("y", [SPC, C, H, W], BF16, kind="ExternalOutput").ap()

    xs = nc.alloc_sbuf_tensor("xs", [C, SPC, HP, HP], BF16).ap()
    wmod = nc.alloc_sbuf_tensor("wmod_sb", [C, SPC, NT * C], BF16).ap()
    outsb = nc.alloc_sbuf_tensor("outsb", [C, NOB, R * W], BF16).ap()
    warm = nc.alloc_sbuf_tensor("warm", [C, 512], BF16).ap()

    cps = [nc.alloc_psum_tensor(f"cps{j}", [C, R * W], F32).ap() for j in range(NPS)]
    wps = nc.alloc_psum_tensor("wps", [C, 512], F32).ap()

    sem_x = [nc.alloc_semaphore(f"sx{i}") for i in range(SPC * NCH)]
    # wmod arrives as 3 tap-triples for sample 0 (so the first conv block can
    # start after the first triple) + one DMA for sample 1
    sem_wm = [nc.alloc_semaphore(f"swm{i}") for i in range(4)]
    sem_pe_blk = nc.alloc_semaphore("pe_blk")
    sem_dve_evict = nc.alloc_semaphore("dve_evict")
    sem_od = nc.alloc_semaphore("sod")   # counting: 16 per output DMA

    with nc.Block() as blk:

        @blk.sync
        def _(eng):
            def xchunk(s, ci):
                r0, r1 = CHUNK_BNDS[ci], CHUNK_BNDS[ci + 1]
                eng.dma_start(
                    out=xs[:, s : s + 1, r0:r1, :],
                    in_=xpad_d[s : s + 1, :, r0:r1, :],
                ).then_inc(sem_x[NCH * s + ci], 16)

            for ci in range(NCH):
                xchunk(0, ci)
            for ci in range(NCH):
                xchunk(1, ci)

        @blk.scalar
        def _(eng):
            # weights ride ACT's own HWDGE queue, in parallel with the
            # x chunks on SP's queue; sample-0 taps split in 3 so the conv
            # can start as soon as the first taps land
            for j in range(3):
                eng.dma_start(
                    out=wmod[:, 0:1, 3 * j * C : 3 * (j + 1) * C],
                    in_=wmod_d[:, 0:1, 3 * j * C : 3 * (j + 1) * C],
                ).then_inc(sem_wm[j], 16)
            eng.dma_start(
                out=wmod[:, 1:2, :], in_=wmod_d[:, 1:2, :]
            ).then_inc(sem_wm[3], 16)

        @blk.tensor
        def _(eng):
            # dummy matmuls on a scratch region (contents irrelevant — results
            # are discarded): keep the PE busy from kernel start so the HAM
            # clock-gate un-throttles before the real stream starts
            for i in range(NWARM):
                eng.matmul(out=wps, lhsT=warm[:, 0:C], rhs=warm[:, 0:512],
                           start=True, stop=True)

            def conv_block(s, b, gb):
                if b == 0 or _chunk_of_block(b) != _chunk_of_block(b - 1):
                    eng.wait_ge(sem_x[NCH * s + _chunk_of_block(b)], 16)
                if gb >= NPS and (gb - NPS) % 4 == 0:
                    # covers bank reuse for blocks gb..gb+3 (reuse distance NPS)
                    eng.wait_ge(sem_dve_evict, gb - NPS + 4)
                for kh in range(KS):
                    for kw in range(KS):
                        t = kh * KS + kw
                        if gb == 0 and t % 3 == 0:
                            eng.wait_ge(sem_wm[t // 3], 16)
                        inst = eng.matmul(
                            out=cps[gb % NPS],
                            lhsT=wmod[:, s : s + 1, t * C : (t + 1) * C],
                            rhs=xs[:, s : s + 1, R * b + kh : R * b + kh + R,
                                   kw : kw + W],
                            start=(t == 0),
                            stop=(t == NT - 1),
                        )
                inst.then_inc(sem_pe_blk, 1)

            for b in range(NB):
                conv_block(0, b, b)
            eng.wait_ge(sem_wm[3], 16)
            for b in range(NB):
                conv_block(1, b, NB + b)

        @blk.vector
        def _(eng):
            # evictions: psum -> sbuf copy (demod scale folded into weights)
            for gb in range(SPC * NB):
                eng.wait_ge(sem_pe_blk, gb + 1)
                if gb >= NOB:
                    eng.wait_ge(sem_od, 16 * (gb - NOB + 1))
                eng.tensor_scalar(outsb[:, gb % NOB : gb % NOB + 1, :],
                                  cps[gb % NPS], 1.0,
                                  None, MULT).then_inc(sem_dve_evict, 1)

        @blk.gpsimd
        def _(eng):
            # output DMAs (GpSimd is otherwise idle; ACT's queue carries wmod)
            for gb in range(SPC * NB):
                s, b = gb // NB, gb % NB
                eng.wait_ge(sem_dve_evict, gb + 1)
                eng.dma_start(
                    out=y_d[s : s + 1, :, R * b : R * b + R, :],
                    in_=outsb[:, gb % NOB : gb % NOB + 1, :],
                ).then_inc(sem_od, 16)

    return nc


def _host_prep(x, w, weight, mod_w, mod_b):
    f = np.float32
    import ml_dtypes

    x = np.asarray(x, f)
    w = np.asarray(w, f)
    weight = np.asarray(weight, f)
    mod_w = np.asarray(mod_w, f)
    mod_b = np.asarray(mod_b, f)

    xpad = np.zeros((B, C, HP, HP), ml_dtypes.bfloat16)
    xpad[:, :, 1 : H + 1, 1 : W + 1] = x.astype(ml_dtypes.bfloat16)

    # style modulation + demodulation folded into the weights on host
    s = (w @ mod_w.T + mod_b).reshape(B, 1, C, 1, 1) + 1.0
    wgt = weight[None] * s                                    # [B, O, I, K, K]
    d = 1.0 / np.sqrt((wgt * wgt).sum(axis=(2, 3, 4)) + 1e-8)  # [B, O]
    wgt = wgt * d[:, :, None, None, None]
    # wmod[i, b, t*C + o] = wgt[b, o, i, kh, kw],  t = kh*3 + kw
    wT = np.ascontiguousarray(wgt.transpose(2, 0, 3, 4, 1)).reshape(C, B, NT * C)
    wT = wT.astype(ml_dtypes.bfloat16)

    in_maps = []
    for core in range(NCORES):
        s0 = SPC * core
        in_maps.append({
            "xpad": np.ascontiguousarray(xpad[s0 : s0 + SPC]),
            "wmod": np.ascontiguousarray(wT[:, s0 : s0 + SPC, :]),
        })
    return in_maps


_cached = {}


def kernel(x, w, weight, mod_w, mod_b):
    if "nc" not in _cached:
        _cached["nc"] = build_program()
    nc = _cached["nc"]
    in_maps = _host_prep(x, w, weight, mod_w, mod_b)
    res = run_bass_kernel_spmd(nc, in_maps, list(range(NCORES)))
    return np.concatenate(
        [res.results[i]["y"].astype(np.float32) for i in range(NCORES)], axis=0)


if __name__ == "__main__":
    from concourse.bass_utils import compile_bass_kernel
    import tempfile

    nc = build_program()
    d = tempfile.mkdtemp()
    neff = compile_bass_kernel(nc, d)
    print("compiled OK:", neff)


# revision 17
# speedup vs baseline: 1.0314x; 1.0314x over previous
"""Modulated conv2d (StyleGAN-2 style, B=16 C=128 HxW=128x128 K=3) on 8 TRN2
NeuronCores, data-parallel over batch (2 samples/core).

v2: the style modulation + demodulation (a ~3 MFLOP computation) is folded
into the weights ON HOST, so the device kernel is a pure grouped conv:

  per core:
    1. DMA: wmod[i, s, t*C+o] = (weight * style * demod)^T  (bf16, 0.59 MB)
            x zero-padded to 130x130 (bf16), in 4 chunks per sample with a
            small first chunk so the conv can start early
    2. warm-up: a few dummy matmuls on a memset scratch region keep the PE
       busy from kernel start so the HAM clock-gate reaches K=8/8 (2.4 GHz)
       before the real conv stream begins (otherwise the first ~3.4 us of
       matmuls run at 1.2 GHz)
    3. conv: per 4-row output block, 9 tap matmuls (K=C_in, M=C_out, N=512)
       accumulate fp32 in PSUM; the tap shift is a strided 3D rhs view into
       the padded image — no im2col; 7 rotating PSUM banks
    4. evict: psum -> sbuf staging copy on DVE (demod scale already in the
       weights), 12 staging buffers
    5. DMA out (ACT-issued HWDGE)

Raw Bass with manual semaphores: this toolchain's walrus accepts only ONE
sync-wait command per instruction, so every engine-pair dependency is guarded
by an explicit single-wait `wait_ge`.

Numerics: bf16 operands, fp32 accumulation; max rel err vs the fp32 jax
reference ~2.2e-3.
"""

import sys

sys.path.insert(0, "/opt/trn_rl_repo")

import numpy as np

import concourse.bass as bass
from concourse import mybir
from concourse.bass_utils import run_bass_kernel_spmd

B, C, H, W, KS, WD = 16, 128, 128, 128, 3, 512
NCORES = 8
SPC = B // NCORES          # samples per core = 2
HP = H + 2                 # padded height/width = 130
NT = KS * KS               # 9 taps

R = 4                      # output rows per conv block (N = R*W = 512; PSUM bank cap)
NPS = 7                    # rotating conv PSUM banks
NOB = 12                   # output staging buffers
NB = H // R                # conv blocks per sample = 32
NWARM = 5                  # PE warm-up matmuls (N=512 each)
CHUNK_BNDS = [0, 10, 18, 50, 90, 130]   # x DMA chunk row boundaries (padded rows)
NCH = len(CHUNK_BNDS) - 1


def _chunk_of_block(b):
    """First x chunk that covers padded rows needed by output block b."""
    need = R * b + R + 1
    for c in range(len(CHUNK_BNDS) - 1):
        if need < CHUNK_BNDS[c + 1]:
            return c
    raise AssertionError


F32 = mybir.dt.float32
BF16 = mybir.dt.bfloat16
MULT = mybir.AluOpType.mult


def build_program():
    nc = bass.Bass(trn_type="TRN2", target_bir_lowering=False, debug=False)

    xpad_d = nc.dram_tensor("xpad", [SPC, C, HP, HP], BF16, kind="ExternalInput").ap()
    wmod_d = nc.dram_tensor("wmod", [C, SPC, NT * C], BF16, kind="ExternalInput").ap()
    y_d = nc.dram_tensor("y", [SPC, C, H, W], BF16, kind="ExternalOutput").ap()

    xs = nc.alloc_sbuf_tensor("xs", [C, SPC, HP, HP], BF16).ap()
    wmod = nc.alloc_sbuf_tensor("wmod_sb", [C, SPC, NT * C], BF16).ap()
    outsb = nc.alloc_sbuf_tensor("outsb", [C, NOB, R * W], BF16).ap()
    warm = nc.alloc_sbuf_tensor("warm", [C, 512], BF16).ap()

    cps = [nc.alloc_psum_tensor(f"cps{j}", [C, R * W], F32).ap() for j in range(NPS)]
    wps = nc.alloc_psum_tensor("wps", [C, 512], F32).ap()

    sem_x = [nc.alloc_semaphore(f"sx{i}") for i in range(SPC * NCH)]
    # wmod arrives as 3 tap-triples for sample 0 (so the first conv block can
    # start after the first triple) + one DMA for sample 1
    sem_wm = [nc.alloc_semaphore(f"swm{i}") for i in range(4)]
    sem_pe_blk = nc.alloc_semaphore("pe_blk")
    sem_dve_evict = nc.alloc_semaphore("dve_evict")
    sem_od = nc.alloc_semaphore("sod")   # counting: 16 per output DMA

    with nc.Block() as blk:

        @blk.sync
        def _(eng):
            def xchunk(s, ci):
                r0, r1 = CHUNK_BNDS[ci], CHUNK_BNDS[ci + 1]
                eng.dma_start(
                    out=xs[:, s : s + 1, r0:r1, :],
                    in_=xpad_d[s : s + 1, :, r0:r1, :],
                ).then_inc(sem_x[NCH * s + ci], 16)

            for ci in range(NCH):
                xchunk(0, ci)
            for ci in range(NCH):
                xchunk(1, ci)

        @blk.scalar
        def _(eng):
            # weights ride ACT's own HWDGE queue, in parallel with the
            # x chunks on SP's queue; sample-0 taps split in 3 so the conv
            # can start as soon as the first taps land
            for j in range(3):
                eng.dma_start(
                    out=wmod[:, 0:1, 3 * j * C : 3 * (j + 1) * C],
                    in_=wmod_d[:, 0:1, 3 * j * C : 3 * (j + 1) * C],
                ).then_inc(sem_wm[j], 16)
            eng.dma_start(
                out=wmod[:, 1:2, :], in_=wmod_d[:, 1:2, :]
            ).then_inc(sem_wm[3], 16)

        @blk.tensor
        def _(eng):
            # dummy matmuls on a scratch region (contents irrelevant — results
            # are discarded): keep the PE busy from kernel start so the HAM
            # clock-gate un-throttles before the real stream starts
            for i in range(NWARM):
                eng.matmul(out=wps, lhsT=warm[:, 0:C], rhs=warm[:, 0:512],
                           start=True, stop=True)

            def conv_block(s, b, gb):
                if b == 0 or _chunk_of_block(b) != _chunk_of_block(b - 1):
                    eng.wait_ge(sem_x[NCH * s + _chunk_of_block(b)], 16)
                if gb >= NPS and (gb - NPS) % 4 == 0:
                    # covers bank reuse for blocks gb..gb+3 (reuse distance NPS)
                    eng.wait_ge(sem_dve_evict, gb - NPS + 4)
                for kh in range(KS):
                    for kw in range(KS):
                        t = kh * KS + kw
                        if gb == 0 and t % 3 == 0:
                            eng.wait_ge(sem_wm[t // 3], 16)
                        inst = eng.matmul(
                            out=cps[gb % NPS],
                            lhsT=wmod[:, s : s + 1, t * C : (t + 1) * C],
                            rhs=xs[:, s : s + 1, R * b + kh : R * b + kh + R,
                                   kw : kw + W],
                            start=(t == 0),
                            stop=(t == NT - 1),
                        )
                inst.then_inc(sem_pe_blk, 1)

            for b in range(NB):
                conv_block(0, b, b)
            eng.wait_ge(sem_wm[3], 16)
            for b in range(NB):
                conv_block(1, b, NB + b)

        @blk.vector
        def _(eng):
            # evictions: psum -> sbuf copy (demod scale folded into weights)
            for gb in range(SPC * NB):
                eng.wait_ge(sem_pe_blk, gb + 1)
                if gb >= NOB:
                    eng.wait_ge(sem_od, 16 * (gb - NOB + 1))
                eng.tensor_scalar(outsb[:, gb % NOB : gb % NOB + 1, :],
                                  cps[gb % NPS], 1.0,
                                  None, MULT).then_inc(sem_dve_evict, 1)

        @blk.scalar
        def _(eng):
            # output DMAs (ACT's queue also carried wmod, which is done by the
            # time the first output block is ready)
            for gb in range(SPC * NB):
                s, b = gb // NB, gb % NB
                eng.wait_ge(sem_dve_evict, gb + 1)
                eng.dma_start(
                    out=y_d[s : s + 1, :, R * b : R * b + R, :],
                    in_=outsb[:, gb % NOB : gb % NOB + 1, :],
                ).then_inc(sem_od, 16)

    return nc


def _host_prep(x, w, weight, mod_w, mod_b):
    f = np.float32
    import ml_dtypes

    x = np.asarray(x, f)
    w = np.asarray(w, f)
    weight = np.asarray(weight, f)
    mod_w = np.asarray(mod_w, f)
    mod_b = np.asarray(mod_b, f)

    xpad = np.zeros((B, C, HP, HP), ml_dtypes.bfloat16)
    xpad[:, :, 1 : H + 1, 1 : W + 1] = x.astype(ml_dtypes.bfloat16)

    # style modulation + demodulation folded into the weights on host
    s = (w @ mod_w.T + mod_b).reshape(B, 1, C, 1, 1) + 1.0
    wgt = weight[None] * s                                    # [B, O, I, K, K]
    d = 1.0 / np.sqrt((wgt * wgt).sum(axis=(2, 3, 4)) + 1e-8)  # [B, O]
    wgt = wgt * d[:, :, None, None, None]
    # wmod[i, b, t*C + o] = wgt[b, o, i, kh, kw],  t = kh*3 + kw
    wT = np.ascontiguousarray(wgt.transpose(2, 0, 3, 4, 1)).reshape(C, B, NT * C)
    wT = wT.astype(ml_dtypes.bfloat16)

    in_maps = []
    for core in range(NCORES):
        s0 = SPC * core
        in_maps.append({
            "xpad": np.ascontiguousarray(xpad[s0 : s0 + SPC]),
            "wmod": np.ascontiguousarray(wT[:, s0 : s0 + SPC, :]),
        })
    return in_maps


_cached = {}


def kernel(x, w, weight, mod_w, mod_b):
    if "nc" not in _cached:
        _cached["nc"] = build_program()
    nc = _cached["nc"]
    in_maps = _host_prep(x, w, weight, mod_w, mod_b)
    res = run_bass_kernel_spmd(nc, in_maps, list(range(NCORES)))
    return np.concatenate(
        [res.results[i]["y"].astype(np.float32) for i in range(NCORES)], axis=0)


if __name__ == "__main__":
    from concourse.bass_utils import compile_bass_kernel
    import tempfile

    nc = build_program()
    d = tempfile.mkdtemp()
    neff = compile_bass_kernel(nc, d)
    print("compiled OK:", neff)


# revision 22
# speedup vs baseline: 1.0837x; 1.0507x over previous
"""Modulated conv2d (StyleGAN-2 style, B=16 C=128 HxW=128x128 K=3) on 8 TRN2
NeuronCores, data-parallel over batch (2 samples/core).

v2: the style modulation + demodulation (a ~3 MFLOP computation) is folded
into the weights ON HOST, so the device kernel is a pure grouped conv:

  per core:
    1. DMA: wmod[i, s, t*C+o] = (weight * style * demod)^T  (bf16, 0.59 MB)
            x zero-padded to 130x130 (bf16), in 4 chunks per sample with a
            small first chunk so the conv can start early
    2. warm-up: a few dummy matmuls on a memset scratch region keep the PE
       busy from kernel start so the HAM clock-gate reaches K=8/8 (2.4 GHz)
       before the real conv stream begins (otherwise the first ~3.4 us of
       matmuls run at 1.2 GHz)
    3. conv: per 4-row output block, 9 tap matmuls (K=C_in, M=C_out, N=512)
       accumulate fp32 in PSUM; the tap shift is a strided 3D rhs view into
       the padded image — no im2col; 7 rotating PSUM banks
    4. evict: psum -> sbuf staging copy on DVE (demod scale already in the
       weights), 12 staging buffers
    5. DMA out (ACT-issued HWDGE)

Raw Bass with manual semaphores: this toolchain's walrus accepts only ONE
sync-wait command per instruction, so every engine-pair dependency is guarded
by an explicit single-wait `wait_ge`.

Numerics: bf16 operands, fp32 accumulation; max rel err vs the fp32 jax
reference ~2.2e-3.
"""

import sys

sys.path.insert(0, "/opt/trn_rl_repo")

import numpy as np

import concourse.bass as bass
from concourse import mybir
from concourse.bass_utils import run_bass_kernel_spmd

B, C, H, W, KS, WD = 16, 128, 128, 128, 3, 512
NCORES = 8
SPC = B // NCORES          # samples per core = 2
HP = H + 2                 # padded height/width = 130
NT = KS * KS               # 9 taps

R = 4                      # output rows per conv block (N = R*W = 512; PSUM bank cap)
NPS = 7                    # rotating conv PSUM banks
NOB = 12                   # output staging buffers
NB = H // R                # conv blocks per sample = 32
NWARM = 7                  # PE warm-up matmuls (N=512 each)
CHUNK_BNDS = [0, 10, 18, 50, 90, 130]   # x DMA chunk row boundaries (padded rows)
NCH = len(CHUNK_BNDS) - 1


def _chunk_of_block(b):
    """First x chunk that covers padded rows needed by output block b."""
    need = R * b + R + 1
    for c in range(len(CHUNK_BNDS) - 1):
        if need < CHUNK_BNDS[c + 1]:
            return c
    raise AssertionError


F32 = mybir.dt.float32
BF16 = mybir.dt.bfloat16
MULT = mybir.AluOpType.mult


def build_program():
    nc = bass.Bass(trn_type="TRN2", target_bir_lowering=False, debug=False)

    xpad_d = nc.dram_tensor("xpad", [SPC, C, HP, HP], BF16, kind="ExternalInput").ap()
    wmod_d = nc.dram_tensor("wmod", [C, SPC, NT * C], BF16, kind="ExternalInput").ap()
    y_d = nc.dram_tensor("y", [SPC, C, H, W], BF16, kind="ExternalOutput").ap()

    xs = nc.alloc_sbuf_tensor("xs", [C, SPC, HP, HP], BF16).ap()
    wmod = nc.alloc_sbuf_tensor("wmod_sb", [C, SPC, NT * C], BF16).ap()
    outsb = nc.alloc_sbuf_tensor("outsb", [C, NOB, R * W], BF16).ap()
    warm = nc.alloc_sbuf_tensor("warm", [C, 512], BF16).ap()

    cps = [nc.alloc_psum_tensor(f"cps{j}", [C, R * W], F32).ap() for j in range(NPS)]
    wps = nc.alloc_psum_tensor("wps", [C, 512], F32).ap()

    sem_x = [nc.alloc_semaphore(f"sx{i}") for i in range(SPC * NCH)]
    sem_wm = [nc.alloc_semaphore(f"swm{i}") for i in range(SPC)]  # wmod halves
    sem_pe_blk = nc.alloc_semaphore("pe_blk")
    sem_dve_evict = nc.alloc_semaphore("dve_evict")
    sem_od = nc.alloc_semaphore("sod")   # counting: 16 per output DMA

    with nc.Block() as blk:

        @blk.sync
        def _(eng):
            def xchunk(s, ci):
                r0, r1 = CHUNK_BNDS[ci], CHUNK_BNDS[ci + 1]
                eng.dma_start(
                    out=xs[:, s : s + 1, r0:r1, :],
                    in_=xpad_d[s : s + 1, :, r0:r1, :],
                ).then_inc(sem_x[NCH * s + ci], 16)

            # single input queue, FIFO: sample-0 weights, then x chunks (the
            # output queue shares the 16 DMA engines — keeping inputs on one
            # early queue avoids starving the critical first transfers)
            eng.dma_start(
                out=wmod[:, 0:1, :], in_=wmod_d[:, 0:1, :]
            ).then_inc(sem_wm[0], 16)
            for ci in range(NCH):
                xchunk(0, ci)
            eng.dma_start(
                out=wmod[:, 1:2, :], in_=wmod_d[:, 1:2, :]
            ).then_inc(sem_wm[1], 16)
            for ci in range(NCH):
                xchunk(1, ci)

        @blk.tensor
        def _(eng):
            # dummy matmuls on a scratch region (contents irrelevant — results
            # are discarded): keep the PE busy from kernel start so the HAM
            # clock-gate un-throttles before the real stream starts
            for i in range(NWARM):
                eng.matmul(out=wps, lhsT=warm[:, 0:C], rhs=warm[:, 0:512],
                           start=True, stop=True)

            def conv_block(s, b, gb):
                if b == 0 or _chunk_of_block(b) != _chunk_of_block(b - 1):
                    eng.wait_ge(sem_x[NCH * s + _chunk_of_block(b)], 16)
                if gb >= NPS and (gb - NPS) % 4 == 0:
                    # covers bank reuse for blocks gb..gb+3 (reuse distance NPS)
                    eng.wait_ge(sem_dve_evict, gb - NPS + 4)
                for kh in range(KS):
                    for kw in range(KS):
                        t = kh * KS + kw
                        inst = eng.matmul(
                            out=cps[gb % NPS],
                            lhsT=wmod[:, s : s + 1, t * C : (t + 1) * C],
                            rhs=xs[:, s : s + 1, R * b + kh : R * b + kh + R,
                                   kw : kw + W],
                            start=(t == 0),
                            stop=(t == NT - 1),
                        )
                inst.then_inc(sem_pe_blk, 1)

            eng.wait_ge(sem_wm[0], 16)
            for b in range(NB):
                conv_block(0, b, b)
            eng.wait_ge(sem_wm[1], 16)
            for b in range(NB):
                conv_block(1, b, NB + b)

        @blk.vector
        def _(eng):
            # evictions: psum -> sbuf copy (demod scale folded into weights)
            for gb in range(SPC * NB):
                eng.wait_ge(sem_pe_blk, gb + 1)
                if gb >= NOB:
                    eng.wait_ge(sem_od, 16 * (gb - NOB + 1))
                eng.tensor_scalar(outsb[:, gb % NOB : gb % NOB + 1, :],
                                  cps[gb % NPS], 1.0,
                                  None, MULT).then_inc(sem_dve_evict, 1)

        @blk.scalar
        def _(eng):
            # output DMAs (ACT's queue also carried wmod, which is done by the
            # time the first output block is ready)
            for gb in range(SPC * NB):
                s, b = gb // NB, gb % NB
                eng.wait_ge(sem_dve_evict, gb + 1)
                eng.dma_start(
                    out=y_d[s : s + 1, :, R * b : R * b + R, :],
                    in_=outsb[:, gb % NOB : gb % NOB + 1, :],
                ).then_inc(sem_od, 16)

    return nc


def _host_prep(x, w, weight, mod_w, mod_b):
    f = np.float32
    import ml_dtypes

    x = np.asarray(x, f)
    w = np.asarray(w, f)
    weight = np.asarray(weight, f)
    mod_w = np.asarray(mod_w, f)
    mod_b = np.asarray(mod_b, f)

    xpad = np.zeros((B, C, HP, HP), ml_dtypes.bfloat16)
    xpad[:, :, 1 : H + 1, 1 : W + 1] = x.astype(ml_dtypes.bfloat16)

    # style modulation + demodulation folded into the weights on host
    s = (w @ mod_w.T + mod_b).reshape(B, 1, C, 1, 1) + 1.0
    wgt = weight[None] * s                                    # [B, O, I, K, K]
    d = 1.0 / np.sqrt((wgt * wgt).sum(axis=(2, 3, 4)) + 1e-8)  # [B, O]
    wgt = wgt * d[:, :, None, None, None]
    # wmod[i, b, t*C + o] = wgt[b, o, i, kh, kw],  t = kh*3 + kw
    wT = np.ascontiguousarray(wgt.transpose(2, 0, 3, 4, 1)).reshape(C, B, NT * C)
    wT = wT.astype(ml_dtypes.bfloat16)

    in_maps = []
    for core in range(NCORES):
        s0 = SPC * core
        in_maps.append({
            "xpad": np.ascontiguousarray(xpad[s0 : s0 + SPC]),
            "wmod": np.ascontiguousarray(wT[:, s0 : s0 + SPC, :]),
        })
    return in_maps


_cached = {}


def kernel(x, w, weight, mod_w, mod_b):
    if "nc" not in _cached:
        _cached["nc"] = build_program()
    nc = _cached["nc"]
    in_maps = _host_prep(x, w, weight, mod_w, mod_b)
    res = run_bass_kernel_spmd(nc, in_maps, list(range(NCORES)))
    return np.concatenate(
        [res.results[i]["y"].astype(np.float32) for i in range(NCORES)], axis=0)


if __name__ == "__main__":
    from concourse.bass_utils import compile_bass_kernel
    import tempfile

    nc = build_program()
    d = tempfile.mkdtemp()
    neff = compile_bass_kernel(nc, d)
    print("compiled OK:", neff)
